# revision 38
# baseline (speedup 1.0000x reference)
"""AlgebraicTransformerLM on 8 Trainium2 NeuronCores (Bass/Tile).

Sharding: tokens 8-way, FOLDED within each 4-core batch group: core g of
a group owns query blocks {g, 7-g} (128 tokens each) of its batch, so
every core has the same causal workload. Transformer layers run with
replicated weights; K/V are AllGathered (f16) within each 4-core group;
the final hidden states are AllGathered across all 8 cores and the LM
head is sharded over vocab (V/8 columns per core).

v6 (2.32ms -> 1.59ms over the v4 baseline):
- folded block assignment makes the causal mask structure identical on
  every core: gathered key position 2t+par covers key block t (par=0)
  or 7-t (par=1). Even positions are computed full-width, odd positions
  only for the high query block -> softmax + AV + their copies drop 25%
  in an SPMD-uniform program; the host un-folds the output rows.
- MAD row sums via PE ones-matmuls accumulated directly into the folded
  [A|B] layout (replaces the strided DVE reduce, no 2-PSUM-input fold).
- attention stages keep the PE FIFO contiguous: all score matmuls
  before the abs-gated MAD matmuls; both heads' AV accumulations before
  the V-latency denominator chains and their broadcasts; gathered keys
  are loaded AND mean-centered one pipeline stage ahead.
- engine rebalance: mask multiply on GpSimd, ReLU+bias and half the
  psum->f16 copies on Scalar, K/V/x collective-consumer DMAs triggered
  from the Scalar queue (gated on the collective by an explicit dep) so
  the Sync queue keeps prefetching weights through the gather window.
- K AllGather split in feature halves: scores for head pairs 0-3 start
  while the second half is still on the wire.
- host pre-packs every weight into its SBUF layout ([128, k*N] planes;
  fg-major for the FFN, q4-major for the LM head) and the gathered V
  carries its denominator ones-columns from the producer side: all big
  DMAs are one contiguous run per partition (the [p,h,ch,e] gathered-V
  load previously issued 16K 128-byte descriptors that stalled the
  queues for ~40us per layer).
- LM head: 512-aligned vocab chunks (odd widths run the PE at ~2GHz
  instead of 2.4GHz), double-buffered weight quarters.
"""

import os
import numpy as np
from dataclasses import dataclass

import ml_dtypes

import concourse.bass as bass
import concourse.mybir as mybir
import concourse.tile as tile
from concourse import bacc
from concourse import bass_utils

from concourse.dve_spec import AluOp as _AluOp, Bin as _Bin, Spec as _Spec, \
    Src0 as _Src0, Src1 as _Src1, C0 as _C0, C1 as _C1, sq as _sq
from concourse import dve_ops as _dve_ops
from concourse.dve_ops import DveOp as _DveOp

_RECIP_C0, _RECIP_C1 = -0.23549792, 2.0017324


def _register_custom_ops():
    if "RAT_RECIP1B" in _dve_ops._SUB_OPCODE_FOR_NAME:
        byname = {op.name: op for op in _dve_ops.OPS}
        return byname["RAT_RECIP1B"], byname["RAT_POW4"]
    # w = one-NR reciprocal of d = |in0| + in1 (in1 = broadcast mad/gain
    # row, folding the score scale into the softsign denominator:
    # t~/(|t~|+1) = t * w).
    _absx = _Bin(_AluOp.ABSOLUTE_VALUE, _Src0, _Src0)
    _d = _absx + _Src1
    _not = _Bin(_AluOp.BITWISE_NOT, _d, _d)
    _y0 = _not * _C0
    _y1 = _y0 * (_C1 - _d * _y0)

    def _ref_ratr(in0, in1, c0, c1, c2):
        import numpy as _np
        d = (_np.abs(in0).astype(_np.float32) + in1).astype(_np.float32)
        nd = (~d.view(_np.int32)).view(_np.float32)
        y0 = nd * c0
        return y0 * (c1 - d * y0)

    ratr = _DveOp("RAT_RECIP1B", _Spec(body=_y1, reference=_ref_ratr),
                  subdim=False,
                  uops_sha={"v3": "3a8e141c460710c6", "v4": "56a2425a5b36d439"})

    _z = _Src0 * _Src1 * _C0 + _C1

    def _ref_ratp(in0, in1, c0, c1, c2):
        import numpy as _np
        z = (in0.astype(_np.float32) * in1 * c0 + c1).astype(_np.float32)
        return (z * z) ** 2

    ratp = _DveOp("RAT_POW4", _Spec(body=_sq(_sq(_z)), reference=_ref_ratp),
                  subdim=False,
                  uops_sha={"v3": "05bc64ec9dd0f8f2", "v4": "a2b1b8b27057ac01"})

    for op in (ratr, ratp):
        _dve_ops.OPS.append(op)
        _dve_ops.CUSTOM_DVE_SPECS[op.name] = op.spec
        _dve_ops._SUB_OPCODE_FOR_NAME[op.name] = (
            max(_dve_ops._SUB_OPCODE_FOR_NAME.values()) + 1)
    return ratr, ratp


RATR_OP, RATP_OP = _register_custom_ops()

f32 = mybir.dt.float32
f32r = mybir.dt.float32r
f16 = mybir.dt.float16
AL = mybir.AluOpType
AF = mybir.ActivationFunctionType
AX = mybir.AxisListType

EPS = 1e-6
NCORES = 8
GROUP = 4  # cores per batch group

# softmax / AV packed column layout, per chunk-pair group (4 groups of
# 512 score cols -> 384 needed cols): [evenA(128) | evenB(128) | oddB(128)]
# (+ oddA(128) computed for MAD only).
GW = 512   # score cols per chunk-pair in PSUM
PW = 384   # packed (needed) cols per chunk-pair


@dataclass
class Cfg:
    L: int = 4
    B: int = 2
    T: int = 1024
    D: int = 1024
    H: int = 16
    F: int = 4096
    V: int = 32000

    @property
    def DH(self):
        return self.D // self.H

    @property
    def TOK(self):
        return self.B * self.T // NCORES  # tokens per core

    @property
    def NTOT(self):
        return self.B * self.T

    @property
    def KT(self):
        return self.D // 128

    @property
    def CH(self):
        return self.T // 128  # kv chunks per batch group

    @property
    def VS(self):
        return self.V // NCORES  # vocab shard per core

    @property
    def VSP(self):
        return self.V // NCORES  # unpadded (padding measured slower)


def _nsplit(total, maxc):
    out = []
    off = 0
    while off < total:
        sz = min(maxc, total - off)
        out.append((off, sz))
        off += sz
    return out


# bias blob column layout (each entry KT=8 cols except b1 = F/128 = 32)
_BLOB = {"n1g": 0, "n2g": 8, "bq": 16, "bk": 24, "bo": 32, "b2": 40,
         "b1": 48}
_BLOB_W = 80


def build(cfg: Cfg, sg_vals, bv_nonzero: bool, blm_nonzero: bool):
    """Build the SPMD program (identical for all 8 cores)."""
    c = cfg
    DH, TOK, KT, CH = c.DH, c.TOK, c.KT, c.CH
    HP = c.H // 2  # head pairs (q/k tiles hold 2 heads of 64 rows)
    assert DH == 64
    assert TOK == 256, "kernel hardcodes 256 tokens/core psum packing"
    TT = TOK // 128
    NG = CH // 2  # chunk-pair groups (4)

    nc = bacc.Bacc(
        "TRN2",
        target_bir_lowering=False,
        debug=False,
        enable_asserts=False,
        num_devices=NCORES,
    )

    def din(name, shape, dt=f32):
        return nc.dram_tensor(name, shape, dt, kind="ExternalInput").ap()

    # weights arrive host-packed in SBUF layout: [128, k*N] where row p,
    # col k*N+n = W[128k+p, n] — each DMA is one contiguous run per
    # partition (few, large descriptors).
    x0 = din("x0", [c.D, TOK])
    wq = din("wq", [c.L, 128, KT * c.D], f16)
    wk = din("wk", [c.L, 128, KT * c.D], f16)
    wv = din("wv", [c.L, 128, KT * c.D], f16)
    wo = din("wo", [c.L, 128, KT * c.D], f16)
    w1 = din("w1", [c.L, 128, KT * c.F], f16)   # fg-major: (fg k n)
    w2 = din("w2", [c.L, 128, (c.F // 128) * c.D], f16)  # (fg k n)
    blob = din("blob", [c.L, 128, _BLOB_W])
    bvT = din("bv", [c.L, c.D])
    fing = din("fing", [128, KT])
    wlm = din("wlm", [128, KT * c.VSP], f16)    # q4-major: (q4 k n)
    blm = din("blm", [c.VSP])
    m01 = din("m01", [128, NG * PW], f16)  # host-packed folded mask

    out = nc.dram_tensor("out", [c.NTOT, c.VSP], f32,
                         kind="ExternalOutput").ap()

    kv_groups = [list(range(GROUP)), list(range(GROUP, NCORES))]
    all_group = [list(range(NCORES))]

    with tile.TileContext(nc) as tc:
        with (
            tc.tile_pool(name="sb", bufs=1) as sb,
            tc.tile_pool(name="ps", bufs=2, space="PSUM") as ps,
            tc.tile_pool(name="dram", bufs=1, space="DRAM") as dram,
        ):
            # ---------------- constants ----------------
            ones_stage = sb.tile([128, 1], f32, name="ones_stage")
            nc.vector.memset(ones_stage[:], 1.0)
            ones_col_b = sb.tile([128, 1], f16, name="ones_col_b")
            nc.vector.tensor_copy(ones_col_b[:], ones_stage[:])
            ones_rstage = sb.tile([1, 128], f32, name="ones_rstage")
            nc.vector.memset(ones_rstage[:], 1.0)
            ones_row = sb.tile([1, 128], f32r, name="ones_row")
            nc.vector.tensor_copy(ones_row[:], ones_rstage[:])
            ones_row32 = ones_rstage
            ones_row_b = sb.tile([1, 128], f16, name="ones_row_b")
            nc.vector.tensor_copy(ones_row_b[:], ones_rstage[:])
            mask_sb = sb.tile([128, NG * PW], f16, name="mask_sb")
            nc.sync.dma_start(mask_sb[:], m01)

            def load_w_half(name, lw, col0, width, tag="w"):
                """[128, width] <- contiguous column slice of a host-packed
                [128, k*N] weight plane."""
                t = sb.tile([128, width], f16, name=name,
                            tag=tag, bufs=4 if tag == "wffn" else 2)
                nc.sync.dma_start(t[:], lw[:, col0:col0 + width])
                return t

            # ---------------- initial x ----------------
            xs_w = sb.tile([128, KT * TOK], f32r, name="xs_w", tag="x",
                           bufs=1)
            nc.sync.dma_start(
                xs_w[:].rearrange("p (k t) -> p k t", k=KT),
                x0.rearrange("(k p) t -> p k t", p=128).bitcast(f32r))
            xs = [xs_w[:, TOK * kt:TOK * (kt + 1)] for kt in range(KT)]

            def anorm(xtiles, gain_cols, tag, out_dt, out_bufs=2):
                """h = gain * x / (mean_D |x| + eps), feature-major.
                Returns a wide [128, KT*TOK] tile. |x| partial sums are
                accumulated on the PE (ones-matmul) instead of a DVE
                add tree."""
                ab = []
                for kt in range(KT):
                    a = sb.tile([128, TOK], f16, name=f"abs_{tag}_{kt}",
                                tag="absx", bufs=3)
                    nc.scalar.activation(a[:], xtiles[kt], AF.Abs)
                    ab.append(a)
                s4 = []
                for j in range(4):
                    s = sb.tile([128, TOK], f16, name=f"as_{tag}_{j}",
                                tag="asum", bufs=5)
                    nc.vector.tensor_tensor(s[:], ab[2 * j][:],
                                            ab[2 * j + 1][:], AL.add)
                    s4.append(s)
                s2 = []
                for j in range(2):
                    s = sb.tile([128, TOK], f16, name=f"as2_{tag}_{j}",
                                tag="asum", bufs=5)
                    nc.vector.tensor_tensor(s[:], s4[2 * j][:],
                                            s4[2 * j + 1][:], AL.add)
                    s2.append(s)
                s1 = sb.tile([128, TOK], f16, name=f"as1_{tag}", tag="asum",
                             bufs=5)
                nc.vector.tensor_tensor(s1[:], s2[0][:], s2[1][:], AL.add)
                mad_ps = ps.tile([128, 512], f32, name=f"mad_{tag}",
                                 tag="mado")
                nc.tensor.matmul(mad_ps[0:1, 0:TOK], ones_col_b[:], s1[:],
                                 start=True, stop=True)
                srow = sb.tile([1, TOK], f32, name=f"srow_{tag}", tag="row",
                               bufs=4)
                nc.vector.tensor_scalar(srow[:], mad_ps[0:1, 0:TOK], 1.0 / c.D,
                                        EPS, AL.mult, AL.add)
                rrow = sb.tile([1, TOK], f32, name=f"rrow_{tag}", tag="row",
                               bufs=4)
                nc.vector.reciprocal_approx_fast(out=rrow[:], in_=srow[:])
                rb_ps = ps.tile([128, 512], f32, name=f"rb_{tag}", tag="mado")
                nc.tensor.matmul(rb_ps[0:128, 0:TOK], ones_row32[:],
                                 rrow[:], start=True, stop=True)
                rb = sb.tile([128, TOK], f32, name=f"rb_{tag}", tag="rb",
                             bufs=2)
                nc.scalar.copy(rb[:], rb_ps[0:128, 0:TOK])
                h_w = sb.tile([128, KT * TOK], out_dt, name=f"h_{tag}",
                              tag="h", bufs=out_bufs)
                for kt in range(KT):
                    nc.vector.scalar_tensor_tensor(
                        h_w[:, TOK * kt:TOK * (kt + 1)], xtiles[kt],
                        gain_cols[:, kt:kt + 1], rb[:], AL.mult, AL.mult)
                return h_w

            def proj_fm(whs, htiles, bias_cols, out_w, m0=0, m1=None):
                """out_w[:, 256m:+256] = W[:, 128m:+128].T @ h + b.
                whs = list of weight half-tiles [128, 4*1024]."""
                for m in range(m0, m1 if m1 is not None else KT):
                    pt = ps.tile([128, 512], f32, name=f"p_{id(out_w)}_{m}",
                                 tag="projgt" if m % 2 == 0 else "mado")
                    for kt in range(KT):
                        wh = whs[kt // 4]
                        col = (kt % 4) * c.D + 128 * m
                        nc.tensor.matmul(
                            pt[0:128, 0:TOK], wh[:, col:col + 128],
                            htiles[kt], start=(kt == 0), stop=(kt == KT - 1))
                    nc.vector.tensor_scalar(out_w[:, TOK * m:TOK * (m + 1)],
                                            pt[0:128, 0:TOK],
                                            bias_cols[:, m:m + 1], None,
                                            AL.add)

            # ================= layers =================
            for l in range(c.L):
                blc = sb.tile([128, _BLOB_W], f32, name=f"blob{l}", tag="bias",
                              bufs=2)
                nc.sync.dma_start(blc[:], blob[l])
                n1c = blc[:, _BLOB["n1g"]:_BLOB["n1g"] + KT]
                n2c = blc[:, _BLOB["n2g"]:_BLOB["n2g"] + KT]
                bqc = blc[:, _BLOB["bq"]:_BLOB["bq"] + KT]
                bkc = blc[:, _BLOB["bk"]:_BLOB["bk"] + KT]
                boc = blc[:, _BLOB["bo"]:_BLOB["bo"] + KT]
                b2c = blc[:, _BLOB["b2"]:_BLOB["b2"] + KT]
                b1c = blc[:, _BLOB["b1"]:_BLOB["b1"] + c.F // 128]

                h1w = anorm(xs, n1c, f"n1l{l}", out_dt=f16)
                h1 = [h1w[:, TOK * kt:TOK * (kt + 1)] for kt in range(KT)]

                # ---- K then V (feed the AllGathers early), Q overlaps AG ----
                wkh = [load_w_half(f"wk{l}_{j}", wk[l], 4 * j * c.D,
                                   4 * c.D) for j in range(2)]
                kfm = sb.tile([128, KT * TOK], f16, name=f"kfm{l}",
                              tag="kfm", bufs=1)
                kh = KT // 2
                kins, ccks, kgds = [], [], []
                for half in range(2):
                    proj_fm(wkh, h1, bkc, kfm, m0=half * kh,
                            m1=(half + 1) * kh)
                    kin = dram.tile([kh * 128, TOK], f16,
                                    name=f"kin{l}_{half}", tag=f"kin{half}",
                                    bufs=2)
                    nc.sync.dma_start(
                        kin[:].rearrange("(k p) t -> p k t", p=128),
                        kfm[:, kh * TOK * half:kh * TOK * (half + 1)]
                        .rearrange("p (k t) -> p k t", k=kh))
                    kg_d = dram.tile([GROUP * kh * 128, TOK], f16,
                                     name=f"kg{l}_{half}", tag=f"kg{half}",
                                     bufs=2)
                    cck = nc.gpsimd.collective_compute(
                        "AllGather", AL.bypass, replica_groups=kv_groups,
                        ins=[kin[:]], outs=[kg_d[:]])
                    kins.append(kin)
                    ccks.append(cck)
                    kgds.append(kg_d)

                wvh = [load_w_half(f"wv{l}_{j}", wv[l], 4 * j * c.D,
                                   4 * c.D) for j in range(2)]
                if bv_nonzero:
                    bvstage = sb.tile([1, c.D], f32, name=f"bvstage{l}",
                                      tag="bvrow", bufs=2)
                    nc.sync.dma_start(bvstage[:], bvT[l:l + 1, :])
                    bvrow = sb.tile([1, c.D], f16, name=f"bvrow{l}",
                                    tag="bvrow2", bufs=2)
                    nc.vector.tensor_copy(bvrow[:], bvstage[:])
                vtm = []
                EW = c.H * (DH + 1)  # 1040: per-token v row incl ones cols
                for mt in range(TT):
                    vt = sb.tile([128, EW], f16, name=f"v{mt}", tag="v",
                                 bufs=2)
                    vt3 = vt[:].rearrange("p (h e) -> p h e", e=DH + 1)
                    nc.gpsimd.memset(vt3[:, :, DH:DH + 1], 1.0)
                    for (noff, nsz) in _nsplit(c.D, 512):
                        pv = ps.tile([128, 512], f32, name=f"pv_{mt}_{noff}",
                                     tag="projgt" if (noff // 512 + mt) % 2
                                     else "mado")
                        for kt in range(KT):
                            wh = wvh[kt // 4]
                            wcol = (kt % 4) * c.D + noff
                            nc.tensor.matmul(
                                pv[:, 0:nsz],
                                h1w[:, TOK * kt + 128 * mt:
                                    TOK * kt + 128 * (mt + 1)],
                                wh[:, wcol:wcol + nsz],
                                start=(kt == 0),
                                stop=(kt == KT - 1) and not bv_nonzero)
                        if bv_nonzero:
                            nc.tensor.matmul(
                                pv[:, 0:nsz], ones_col_b[:],
                                bvrow[0:1, noff:noff + nsz],
                                start=False, stop=True)
                        h0 = noff // DH
                        nc.scalar.copy(vt3[:, h0:h0 + nsz // DH, 0:DH],
                                       pv[:, 0:nsz].rearrange(
                                           "p (h e) -> p h e", e=DH))
                    vtm.append(vt)
                vin = dram.tile([TOK, EW], f16, name=f"vin{l}", tag="vin",
                                bufs=2)
                for mt in range(TT):
                    nc.sync.dma_start(vin[128 * mt:128 * (mt + 1), :],
                                      vtm[mt][:])
                vg_d = dram.tile([GROUP * TOK, EW], f16, name=f"vg{l}",
                                 tag="vg", bufs=2)
                cc_v = nc.gpsimd.collective_compute(
                    "AllGather", AL.bypass, replica_groups=kv_groups,
                    ins=[vin[:]], outs=[vg_d[:]])

                wqh = [load_w_half(f"wq{l}_{j}", wq[l], 4 * j * c.D,
                                   4 * c.D) for j in range(2)]
                qfm = sb.tile([128, KT * TOK], f16, name=f"qfm{l}",
                              tag="qfm", bufs=1)
                proj_fm(wqh, h1, bqc, qfm)

                # hoist Wo + first-FFN-group weight loads above the barrier
                woh = [load_w_half(f"wo{l}_{j}", wo[l], 4 * j * c.D,
                                   4 * c.D) for j in range(2)]
                FGW = min(c.F, 1024)
                FG = c.F // FGW
                FGT = FGW // 128
                w1pre = [load_w_half(f"w1{l}_0_{j}", w1[l],
                                     4 * j * FGW, 4 * FGW, tag="wffn")
                         for j in range(2)]


                # ---- gathered V (+ ones columns), one contiguous DMA ----
                vga = sb.tile([128, CH * EW], f16, name=f"vga{l}",
                              tag="vga", bufs=1)
                vga_dma = nc.scalar.dma_start(
                    vga[:].rearrange("p (ch w) -> p ch w", ch=CH),
                    vg_d[:].rearrange("(ch p) w -> p ch w", p=128))
                tile.add_dep_helper(vga_dma.ins, cc_v.ins, sync=True,
                                    reason="vga load waits on V AG")

                # ---- attention: software-pipelined head pairs ----
                ofm = sb.tile([128, KT * TOK], f16, name=f"ofm{l}",
                              tag="ofm", bufs=1)
                st = [dict() for _ in range(HP)]

                def prep_kgt(p, l=l):
                    """Load + mean-center the gathered keys for head pair
                    p, one pipeline stage ahead of its score matmuls (the
                    centering rides the deep DVE queue)."""
                    s = st[p]
                    kgt = sb.tile([128, GROUP * TOK], f16, name=f"kgt{l}_{p}",
                                  tag="kgt", bufs=3)
                    half, ph = p // (HP // 2), p % (HP // 2)
                    kgt_dma = nc.scalar.dma_start(
                        kgt[:].rearrange("p (r t) -> p r t", r=GROUP),
                        kgds[half][:].rearrange("(r k p) t -> k p r t",
                                                r=GROUP, p=128)[ph])
                    tile.add_dep_helper(kgt_dma.ins, ccks[half].ins,
                                        sync=True,
                                        reason="kgt load waits on K AG")
                    ks = sb.tile([128, 1], f32, name=f"ksum{p}", tag="ksum",
                                 bufs=2)
                    nc.scalar.activation(kgt[:], kgt[:], AF.Copy,
                                         accum_out=ks[:])
                    nc.vector.tensor_scalar(ks[:], ks[:], -1.0 / c.T, None,
                                            AL.mult)
                    nc.vector.tensor_scalar(kgt[:], kgt[:], ks[:, 0:1],
                                            None, AL.add)
                    s["kgt"] = kgt

                def stage_scores(p, l=l):
                    s = st[p]
                    kgt = s["kgt"]
                    s["at"] = {}
                    s["tt"] = {}
                    s["gv"] = {}
                    for hf in range(2):
                        rsl = slice(64 * hf, 64 * hf + 64)
                        qh = qfm[rsl, TOK * p:TOK * (p + 1)]
                        qA = qfm[rsl, TOK * p:TOK * p + 128]
                        qB = qfm[rsl, TOK * p + 128:TOK * (p + 1)]
                        at_w = sb.tile([128, CH * TOK], f16,
                                       name=f"at{p}_{hf}", tag="at", bufs=3)
                        tt_w = sb.tile([128, CH * TOK], f16,
                                       name=f"tt{p}_{hf}", tag="tt", bufs=3)
                        for u in range(2):
                            spt = ps.tile([128, 1024], f32,
                                          name=f"sp{p}_{hf}_{u}", tag="sc")
                            for v in range(2):
                                t2 = 2 * u + v
                                base = GW * v
                                ke = kgt[rsl, 128 * (2 * t2):
                                         128 * (2 * t2) + 128]
                                ko = kgt[rsl, 128 * (2 * t2 + 1):
                                         128 * (2 * t2 + 1) + 128]
                                # [evenA evenB | oddA oddB]: odd halves in
                                # A,B order so one MAD matmul folds both
                                nc.tensor.matmul(
                                    spt[:, base:base + 256], ke, qh,
                                    start=True, stop=True)
                                nc.tensor.matmul(
                                    spt[:, base + 256:base + 384], ko, qA,
                                    start=True, stop=True)
                                nc.tensor.matmul(
                                    spt[:, base + 384:base + 512], ko, qB,
                                    start=True, stop=True)
                            nc.scalar.activation(
                                at_w[:, 1024 * u:1024 * (u + 1)], spt[:],
                                AF.Abs)
                            # signed scores (full width: contiguous copy
                            # is cheaper than a strided packed one)
                            tdst = tt_w[:, 1024 * u:1024 * (u + 1)]
                            nc.scalar.copy(tdst, spt[:])
                        s["at"][hf] = at_w
                        s["tt"][hf] = tt_w
                    # MAD after both heads' score matmuls so the PE FIFO
                    # does not block on Scalar's abs mid-stream. Both the
                    # even [eA eB] and odd [oA oB] halves fold straight
                    # onto madp[0:256] = per-token [A(128) B(128)] sums.
                    for hf in range(2):
                        at_w = s["at"][hf]
                        madp = ps.tile([128, 512], f32, name=f"madp{p}_{hf}",
                                       tag="mado")
                        for t2 in range(4):
                            nc.tensor.matmul(
                                madp[0:1, 0:256], ones_col_b[:],
                                at_w[:, GW * t2:GW * t2 + 256],
                                start=(t2 == 0), stop=False)
                            nc.tensor.matmul(
                                madp[0:1, 0:256], ones_col_b[:],
                                at_w[:, GW * t2 + 256:GW * (t2 + 1)],
                                start=False, stop=(t2 == 3))
                        sg = float(sg_vals[l])
                        ginv = sb.tile([1, 256], f16, name=f"ginv{p}_{hf}",
                                       tag="row2", bufs=4)
                        nc.vector.tensor_scalar(
                            ginv[0:1, 0:256], madp[0:1, 0:256],
                            1.0 / (c.T * sg), EPS / sg, AL.mult, AL.add)
                        s["gv"][hf] = ginv

                def stage_softmax(p):
                    s = st[p]
                    s["ph"] = {}
                    for hf in range(2):
                        at_w = s["at"][hf]
                        tt_w = s["tt"][hf]
                        ginv = s["gv"][hf]
                        gt_ps = ps.tile([128, 512], f32, name=f"gt{p}_{hf}",
                                        tag="projgt")
                        nc.tensor.matmul(gt_ps[0:128, 0:256], ones_row_b[:],
                                         ginv[:], start=True, stop=True)
                        at3 = at_w[:].rearrange("p (g c) -> p g c", g=NG)
                        tt3 = tt_w[:].rearrange("p (g c) -> p g c", g=NG)
                        rr_w = sb.tile([128, NG * PW], f16,
                                       name=f"rr{p}_{hf}", tag="rr", bufs=2)
                        rr3 = rr_w[:].rearrange("p (g c) -> p g c", g=NG)
                        ph_w = sb.tile([128, NG * PW], f16,
                                       name=f"ph{p}_{hf}", tag="scr", bufs=3)
                        ph3 = ph_w[:].rearrange("p (g c) -> p g c", g=NG)
                        nc.vector._custom_dve(
                            RATR_OP, out=rr3[:, :, 0:256],
                            in0=at3[:, :, 0:256],
                            in1=gt_ps[0:128, 0:256].unsqueeze(1)
                                .broadcast_to([128, NG, 256]),
                            s0=_RECIP_C0, s1=_RECIP_C1)
                        nc.vector._custom_dve(
                            RATR_OP, out=rr3[:, :, 256:384],
                            in0=at3[:, :, 384:512],
                            in1=gt_ps[0:128, 128:256].unsqueeze(1)
                                .broadcast_to([128, NG, 128]),
                            s0=_RECIP_C0, s1=_RECIP_C1)
                        nc.vector._custom_dve(
                            RATP_OP, out=ph3[:, :, 0:256],
                            in0=tt3[:, :, 0:256], in1=rr3[:, :, 0:256],
                            s0=0.5, s1=0.5)
                        nc.vector._custom_dve(
                            RATP_OP, out=ph3[:, :, 256:384],
                            in0=tt3[:, :, 384:512], in1=rr3[:, :, 256:384],
                            s0=0.5, s1=0.5)
                        nc.gpsimd.tensor_tensor(ph_w[:, 0:2 * PW],
                                                ph_w[:, 0:2 * PW],
                                                mask_sb[:, 0:2 * PW], AL.mult)
                        nc.gpsimd.tensor_tensor(ph_w[:, 2 * PW:],
                                                ph_w[:, 2 * PW:],
                                                mask_sb[:, 2 * PW:], AL.mult)
                        s["ph"][hf] = ph_w

                def stage_av(p):
                    s = st[p]
                    ops2 = {}
                    for hf in range(2):
                        h = 2 * p + hf
                        ph_w = s["ph"][hf]
                        o_ps = ps.tile([128, 512], f32, name=f"ops{h}",
                                       tag="mado")
                        ops2[hf] = o_ps
                        for t2 in range(4):
                            che = 2 * t2
                            cho = 2 * t2 + 1
                            nc.tensor.matmul(
                                o_ps[0:DH + 1, 0:TOK],
                                vga[:, che * EW + h * (DH + 1):
                                    che * EW + (h + 1) * (DH + 1)],
                                ph_w[:, PW * t2:PW * t2 + 256],
                                start=(t2 == 0), stop=False)
                            nc.tensor.matmul(
                                o_ps[0:DH + 1, 128:256],
                                vga[:, cho * EW + h * (DH + 1):
                                    cho * EW + (h + 1) * (DH + 1)],
                                ph_w[:, PW * t2 + 256:PW * t2 + 384],
                                start=False, stop=(t2 == 3))
                    drrs = {}
                    for hf in range(2):
                        h = 2 * p + hf
                        o_ps = ops2[hf]
                        dr = sb.tile([1, TOK], f32, name=f"dr{h}", tag="row",
                                     bufs=4)
                        nc.vector.tensor_scalar(dr[:], o_ps[DH:DH + 1, 0:TOK],
                                                EPS, None, AL.add)
                        drr = sb.tile([1, TOK], f32, name=f"drr{h}",
                                      tag="row", bufs=4)
                        nc.vector.reciprocal_approx_fast(out=drr[:], in_=dr[:])
                        drrs[hf] = drr
                    for hf in range(2):
                        h = 2 * p + hf
                        o_ps = ops2[hf]
                        nc.tensor.matmul(o_ps[0:DH, 256:256 + TOK],
                                         ones_row32[0:1, 0:DH],
                                         drrs[hf][:], start=True, stop=True)
                        rdb = sb.tile([DH, TOK], f32, name=f"rdb{h}",
                                      tag="rdb", bufs=2)
                        nc.scalar.copy(rdb[:], o_ps[0:DH, 256:256 + TOK])
                        nc.vector.tensor_tensor(
                            ofm[64 * hf:64 * hf + 64, TOK * p:TOK * (p + 1)],
                            o_ps[0:DH, 0:TOK], rdb[:], AL.mult)

                prep_kgt(0)
                for p in range(HP):
                    stage_scores(p)
                    if p + 1 < HP:
                        prep_kgt(p + 1)
                    if p >= 1:
                        stage_softmax(p - 1)
                    if p >= 2:
                        stage_av(p - 2)
                stage_softmax(HP - 1)
                stage_av(HP - 2)
                stage_av(HP - 1)

                # ---- Wo + residual (in place on x) ----
                for m in range(KT):
                    pd = ps.tile([128, 512], f32, name=f"pwo_{m}",
                                 tag="projgt" if m % 2 == 0 else "mado")
                    for kt in range(KT):
                        wh = woh[kt // 4]
                        col = (kt % 4) * c.D + 128 * m
                        nc.tensor.matmul(pd[0:128, 0:TOK],
                                         wh[:, col:col + 128],
                                         ofm[:, TOK * kt:TOK * (kt + 1)],
                                         start=(kt == 0), stop=(kt == KT - 1))
                    nc.vector.scalar_tensor_tensor(
                        xs[m], pd[0:128, 0:TOK], boc[:, m:m + 1], xs[m],
                        AL.add, AL.add)

                # ---- FFN ----
                h2w = anorm(xs, n2c, f"n2l{l}", out_dt=f16)
                h2 = [h2w[:, TOK * kt:TOK * (kt + 1)] for kt in range(KT)]
                for fg in range(FG):
                    if fg == 0:
                        w1h = w1pre
                    else:
                        w1h = [load_w_half(
                            f"w1{l}_{fg}_{j}", w1[l],
                            fg * KT * FGW + 4 * j * FGW, 4 * FGW,
                            tag="wffn") for j in range(2)]
                    w2h = [load_w_half(
                        f"w2{l}_{fg}_{j}", w2[l],
                        fg * FGT * c.D + 4 * j * c.D, 4 * c.D,
                        tag="wffn") for j in range(2)]
                    u_w = sb.tile([128, FGT * TOK], f16, name=f"u_{fg}",
                                  tag="uffn", bufs=1)
                    for m in range(FGT):
                        pu = ps.tile([128, 512], f32, name=f"pu_{fg}_{m}",
                                     tag="projgt" if m % 2 == 0 else "mado")
                        for kt in range(KT):
                            wh = w1h[kt // 4]
                            col = (kt % 4) * FGW + 128 * m
                            nc.tensor.matmul(
                                pu[0:128, 0:TOK], wh[:, col:col + 128],
                                h2[kt], start=(kt == 0), stop=(kt == KT - 1))
                        bcol = (FGW * fg) // 128 + m
                        nc.scalar.activation(
                            u_w[:, TOK * m:TOK * (m + 1)], pu[0:128, 0:TOK],
                            AF.Relu, bias=b1c[:, bcol:bcol + 1])
                    for m2 in range(KT):
                        pdl = ps.tile([128, 512], f32, name=f"pdl_{fg}_{m2}",
                                      tag="projgt" if m2 % 2 == 0
                                      else "mado")
                        for ktl in range(FGT):
                            wh = w2h[ktl // 4]
                            col = (ktl % 4) * c.D + 128 * m2
                            nc.tensor.matmul(
                                pdl[0:128, 0:TOK], wh[:, col:col + 128],
                                u_w[:, TOK * ktl:TOK * (ktl + 1)],
                                start=(ktl == 0), stop=(ktl == FGT - 1))
                        if fg == 0:
                            nc.vector.scalar_tensor_tensor(
                                xs[m2], pdl[0:128, 0:TOK],
                                b2c[:, m2:m2 + 1], xs[m2], AL.add, AL.add)
                        else:
                            nc.vector.tensor_tensor(
                                xs[m2], pdl[0:128, 0:TOK], xs[m2], AL.add)

            # ============== final norm + AllGather + LM head ==============
            finc = sb.tile([128, KT], f32, name="finc", tag="bias", bufs=2)
            nc.sync.dma_start(finc[:], fing)
            xfw = anorm(xs, finc, "fin", out_dt=f16)
            xf_in = dram.tile([c.D, TOK], f16, name="xf_in")
            nc.sync.dma_start(
                xf_in[:].rearrange("(k p) t -> p k t", p=128),
                xfw[:].rearrange("p (k t) -> p k t", k=KT))
            xg_d = dram.tile([NCORES * c.D, TOK], f16, name="xg_d",
                             addr_space="Shared")
            cc_x = nc.gpsimd.collective_compute(
                "AllGather", AL.bypass, replica_groups=all_group,
                ins=[xf_in[:]], outs=[xg_d[:]])
            # prefetch first LM-head weight group before the barrier
            NQ = 4
            QW = c.VSP // NQ
            nchunks = _nsplit(QW, 512)

            def load_wlm(q4):
                t = sb.tile([128, KT * QW], f16, name=f"wlm_{q4}",
                            tag="wlm", bufs=2)
                nc.sync.dma_start(
                    t[:], wlm[:, KT * QW * q4:KT * QW * (q4 + 1)])
                return t

            wl0 = load_wlm(0)

            if blm_nonzero:
                blmstage = sb.tile([1, c.VSP], f32, name="blmstage")
                nc.sync.dma_start(blmstage[:], blm[None, :])
                blmrow = sb.tile([1, c.VSP], f16, name="blmrow")
                nc.vector.tensor_copy(blmrow[:], blmstage[:])

            for q4 in range(NQ):
                wlt = wl0 if q4 == 0 else load_wlm(q4)
                for r in range(NCORES):
                    xgt = sb.tile([128, KT * TOK], f16, name=f"xg_{r}",
                                  tag="xg", bufs=2)
                    xgt_dma = nc.scalar.dma_start(
                        xgt[:].rearrange("p (k t) -> p k t", k=KT),
                        xg_d[c.D * r:c.D * (r + 1), :]
                        .rearrange("(k p) t -> p k t", p=128))
                    tile.add_dep_helper(xgt_dma.ins, cc_x.ins, sync=True,
                                        reason="xgt load waits on x AG")
                    for mloc in range(TT):
                        mt = r * TT + mloc
                        for (noff, nsz) in nchunks:
                            pl = ps.tile([128, 512], f32,
                                         name=f"plm_{mt}_{noff}",
                                         tag="projgt" if (noff // 500) % 2
                                         else "mado")
                            for kt in range(KT):
                                nc.tensor.matmul(
                                    pl[0:128, 0:nsz],
                                    xgt[:, TOK * kt + 128 * mloc:
                                        TOK * kt + 128 * (mloc + 1)],
                                    wlt[:, QW * kt + noff:QW * kt + noff + nsz],
                                    start=(kt == 0),
                                    stop=(kt == KT - 1) and not blm_nonzero)
                            if blm_nonzero:
                                nc.tensor.matmul(
                                    pl[0:128, 0:nsz], ones_col_b[:],
                                    blmrow[0:1,
                                           QW * q4 + noff:QW * q4 + noff + nsz],
                                    start=False, stop=True)
                            osb = sb.tile([128, 512], f32,
                                          name=f"olm_{mt}_{noff}",
                                          tag="olm", bufs=2)
                            nc.scalar.copy(osb[:, 0:nsz], pl[0:128, 0:nsz])
                            nc.sync.dma_start(
                                out[128 * mt:128 * (mt + 1),
                                    QW * q4 + noff:QW * q4 + noff + nsz],
                                osb[:, 0:nsz])

    nc.compile()
    return nc


# --------------------------------------------------------------------------
# host wrapper
# --------------------------------------------------------------------------

_CACHE = {}


def _get_nc(cfg, sg_vals, bv_nz, blm_nz):
    key = (str(vars(cfg)), tuple(np.asarray(sg_vals, np.float32).tolist()),
           bv_nz, blm_nz)
    if key not in _CACHE:
        _CACHE[key] = build(cfg, sg_vals, bv_nz, blm_nz)
    return _CACHE[key]


def _pack_cols(v, n):
    """[..., n] -> [..., 128, n//128] with col k = v[..., 128k:128k+128]."""
    v = np.asarray(v, np.float32)
    shp = v.shape[:-1]
    return np.ascontiguousarray(
        v.reshape(shp + (n // 128, 128)).swapaxes(-1, -2))


def _block_order():
    """Global 128-token block ids in core order (folded assignment)."""
    order = []
    for core in range(NCORES):
        b = core // GROUP
        g = core % GROUP
        order.append(b * 8 + g)
        order.append(b * 8 + (7 - g))
    return order


def make_in_maps(cfg, inputs):
    c = cfg
    idx = np.asarray(inputs["idx"])
    tok_emb = np.asarray(inputs["tok_emb"], dtype=np.float32)
    pos_emb = np.asarray(inputs["pos_emb"], dtype=np.float32)
    x_full = tok_emb[idx] + pos_emb[None, :c.T, :]
    x_flat = x_full.reshape(c.NTOT, c.D)

    def _pack_sq(w):  # [L, 1024, N] -> [L, 128, 8*N] (k-major cols)
        L_, Din, N = w.shape
        return np.ascontiguousarray(
            w.reshape(L_, Din // 128, 128, N).transpose(0, 2, 1, 3)
            .reshape(L_, 128, (Din // 128) * N))

    shared = {}
    for k, v in (("wq", "Wq"), ("wk", "Wk"), ("wv", "Wv"), ("wo", "Wo")):
        shared[k] = _pack_sq(
            np.asarray(inputs[v], np.float32).astype(np.float16))
    w1f = np.asarray(inputs["W1"], np.float32).astype(np.float16)
    # [L, 1024, 4096] -> [L, 128, (fg k n)] with fg=4, k=8, n=1024
    shared["w1"] = np.ascontiguousarray(
        w1f.reshape(c.L, 8, 128, 4, 1024).transpose(0, 2, 3, 1, 4)
        .reshape(c.L, 128, 32768))
    w2f = np.asarray(inputs["W2"], np.float32).astype(np.float16)
    # [L, 4096, 1024] -> [L, 128, (fg kk n)] with fg=4, kk=8, n=1024
    shared["w2"] = np.ascontiguousarray(
        w2f.reshape(c.L, 4, 8, 128, 1024).transpose(0, 3, 1, 2, 4)
        .reshape(c.L, 128, 32768))
    blob = np.zeros((c.L, 128, _BLOB_W), np.float32)
    for nm, src in [("n1g", "norm1_gain"), ("n2g", "norm2_gain"),
                    ("bq", "bq"), ("bk", "bk"), ("bo", "bo"), ("b2", "b2"),
                    ("b1", "b1")]:
        v = np.asarray(inputs[src], np.float32)
        p = _pack_cols(v, v.shape[-1])
        blob[:, :, _BLOB[nm]:_BLOB[nm] + p.shape[-1]] = p
    shared["blob"] = blob
    shared["fing"] = _pack_cols(inputs["final_gain"], c.D)
    shared["bv"] = np.ascontiguousarray(np.asarray(inputs["bv"], np.float32))
    wlm_full = np.asarray(inputs["Wlm"], np.float32)
    blm_full = np.asarray(inputs["blm"], np.float32)

    # packed folded causal mask, per core: 4 groups of
    # [evenA(128) | evenB(128) | oddB(128)] where group t covers key
    # blocks t (even position) and 7-t (odd position); query blocks are
    # A = g, B = 7 - g (block-local causal: tri within equal blocks).
    tri = (np.arange(128)[:, None] <= np.arange(128)[None, :])
    ones = np.ones((128, 128), bool)
    zeros = np.zeros((128, 128), bool)

    in_maps = []
    for core in range(NCORES):
        g = core % GROUP
        b = core // GROUP
        blkA, blkB = g, 7 - g
        rows = np.r_[b * c.T + blkA * 128:b * c.T + blkA * 128 + 128,
                     b * c.T + blkB * 128:b * c.T + blkB * 128 + 128]
        m = dict(shared)
        m["x0"] = np.ascontiguousarray(x_flat[rows, :].T)
        wlm_c = wlm_full[:, core * c.VS:(core + 1) * c.VS] \
            .astype(np.float16)
        qw = c.VSP // 4
        # [1024, VS] -> [128, (q4 k n)] with q4=4, k=8, n=qw
        m["wlm"] = np.ascontiguousarray(
            wlm_c.reshape(8, 128, 4, qw).transpose(1, 2, 0, 3)
            .reshape(128, 8 * c.VSP))
        m["blm"] = np.ascontiguousarray(
            blm_full[core * c.VS:(core + 1) * c.VS])
        groups = []
        for t in range(4):
            kbe, kbo = t, 7 - t
            mA = ones if kbe < blkA else (tri if kbe == blkA else zeros)
            mB = ones if kbe < blkB else (tri if kbe == blkB else zeros)
            mOB = ones if kbo < blkB else (tri if kbo == blkB else zeros)
            groups.append(np.concatenate([mA, mB, mOB], axis=1))
        m["m01"] = np.ascontiguousarray(
            np.concatenate(groups, axis=1).astype(np.float16))
        in_maps.append(m)
    return in_maps


LAST_RESULTS = None


def _trace_ready():
    """Tracing needs the axon NTFF hook; register it if possible."""
    try:
        import sys
        import types
        import antenv
        if "antenv.axon_hooks" not in sys.modules:
            mod = types.ModuleType("antenv.axon_hooks")
            state = {"hook": None}
            mod.set_axon_ntff_profile_hook = lambda h: state.update(hook=h)
            mod.get_axon_ntff_profile_hook = lambda: state["hook"]
            sys.modules["antenv.axon_hooks"] = mod
            antenv.axon_hooks = mod
            from trn_agent_boot.trn_boot import _ntff_profile_via_ctypes
            hook = _ntff_profile_via_ctypes("/opt/axon/libaxon_pjrt.so")
            if hook is None:
                return False
            mod.set_axon_ntff_profile_hook(hook)
        return True
    except Exception:
        return False


def kernel(**inputs):
    global LAST_RESULTS
    cfg = Cfg()
    sg = np.asarray(inputs["score_gain"], np.float32)
    bv_nz = bool(np.any(np.asarray(inputs["bv"])))
    blm_nz = bool(np.any(np.asarray(inputs["blm"])))
    nc = _get_nc(cfg, sg, bv_nz, blm_nz)
    in_maps = make_in_maps(cfg, inputs)
    do_trace = (os.environ.get("BASS_TRACE", "") == "1") and _trace_ready()
    if not do_trace:
        # run_bass_kernel_spmd re-reads BASS_TRACE from the environment;
        # make an un-traceable environment safe.
        os.environ["BASS_NEVER_TRACE"] = "1"
    res = bass_utils.run_bass_kernel_spmd(
        nc, in_maps, core_ids=list(range(NCORES)),
        trace=do_trace)
    LAST_RESULTS = res
    outs = [res.results[i]["out"][:, :cfg.VS] for i in range(NCORES)]
    full = np.concatenate(outs, axis=1)  # [NTOT (folded order), V]
    # un-fold: rows are 128-blocks in _block_order(); restore global order
    order = _block_order()
    inv = np.argsort(order)
    full = full.reshape(16, 128, cfg.V)[inv].reshape(cfg.NTOT, cfg.V)
    return full.reshape(cfg.B, cfg.T, cfg.V).astype(np.float32)
